# revision 5
# baseline (speedup 1.0000x reference)
"""AWD-LSTM Trainium2 kernel: 8-core SPMD, gate-sharded LSTM scan with
per-step AllGather h-exchange, vocab-sharded tied-embedding logits.

Self-contained; shapes hardcoded: B=32, T=128, V=32000, E=400, M=512,
H=1152 (layers 0,1), H2=400 (layer 2). Token order on device is t-major
(n = t*B + b) so per-layer input GEMMs block-pipeline behind the previous
layer's scan (wavefront emission).
"""
import sys
import numpy as np

sys.path.insert(0, "/opt/trn_rl_repo")

B, T, V, E, M, H = 32, 128, 32000, 400, 512, 1152
H2 = 400
NC = 8
S01 = H // NC
S2 = H2 // NC
VSH = V // NC
NT = B * T
NMT = NT // 128

_COMPILED = None


def _pack_kT(w, dtype=np.float32):
    """[K, M] -> [128, ceil(K/128)*M]; tile kt at [:, kt*M:(kt+1)*M]."""
    K, Mw = w.shape
    nkt = (K + 127) // 128
    out = np.zeros((128, nkt * Mw), dtype)
    for kt in range(nkt):
        k0, k1 = kt * 128, min(K, (kt + 1) * 128)
        out[: k1 - k0, kt * Mw:(kt + 1) * Mw] = w[k0:k1]
    return out


def _bf16(x):
    import jax.numpy as jnp
    return np.asarray(jnp.asarray(np.asarray(x), dtype=jnp.bfloat16))


def _build():
    import concourse.bass as bass
    import concourse.bacc as bacc
    import concourse.tile as tile
    from concourse import mybir
    from concourse.bass import IndirectOffsetOnAxis

    f32, bf16, i32 = mybir.dt.float32, mybir.dt.bfloat16, mybir.dt.int32
    AF = mybir.ActivationFunctionType
    Alu = mybir.AluOpType

    nc = bacc.Bacc("TRN2", target_bir_lowering=False, debug=False, num_devices=NC)

    ids = nc.dram_tensor("ids", [128, NMT], i32, kind="ExternalInput")
    embt = nc.dram_tensor("embt", [V, E], f32, kind="ExternalInput")
    dW = nc.dram_tensor("dW", [128, 4 * M], bf16, kind="ExternalInput")
    db = nc.dram_tensor("db", [128, 4], f32, kind="ExternalInput")
    wihT, whhT = {}, {}
    for l, (kin, sg) in enumerate([(4, S01), (9, S01), (9, S2)]):
        wihT[l] = nc.dram_tensor(f"wihT{l}", [128, kin * 4 * sg], bf16,
                                 kind="ExternalInput")
    for l, (kh, sg) in enumerate([(9, S01), (9, S01), (4, S2)]):
        whhT[l] = nc.dram_tensor(f"whhT{l}", [128, kh * 4 * sg], bf16,
                                 kind="ExternalInput")
    bias = {l: nc.dram_tensor(f"bias{l}", [128, 4 * sg], f32,
                              kind="ExternalInput")
            for l, sg in [(0, S01), (1, S01), (2, S2)]}
    h0T = {l: nc.dram_tensor(f"h0T{l}", [128, kh * 32], f32,
                             kind="ExternalInput")
           for l, kh in [(0, 9), (1, 9), (2, 4)]}
    c0s = {l: nc.dram_tensor(f"c0s{l}", [32, sg], f32, kind="ExternalInput")
           for l, sg in [(0, S01), (1, S01), (2, S2)]}
    lembT = nc.dram_tensor("lembT", [128, 4 * VSH], bf16, kind="ExternalInput")
    ident = nc.dram_tensor("ident", [128, 128], bf16, kind="ExternalInput")

    logits = nc.dram_tensor("logits", [NT, VSH], f32, kind="ExternalOutput")
    hf = {l: nc.dram_tensor(f"hf{l}", [32, sg], f32, kind="ExternalOutput")
          for l, sg in [(0, S01), (1, S01), (2, S2)]}
    cf = {l: nc.dram_tensor(f"cf{l}", [32, sg], f32, kind="ExternalOutput")
          for l, sg in [(0, S01), (1, S01), (2, S2)]}

    KIN = {0: 4, 1: 9, 2: 9}
    KH = {0: 9, 1: 9, 2: 4}
    KHV = {0: 128, 1: 128, 2: 16}
    SG = {0: S01, 1: S01, 2: S2}
    G4 = {l: 4 * SG[l] for l in range(3)}

    with tile.TileContext(nc) as tc:
        with (
            tc.tile_pool(name="wpool", bufs=1) as wp,
            tc.tile_pool(name="fe", bufs=2) as fe,
            tc.tile_pool(name="scan", bufs=2) as sp,
            tc.tile_pool(name="lg", bufs=5) as lg,
            tc.tile_pool(name="psA", bufs=1, space="PSUM") as psA,
            tc.tile_pool(name="psB", bufs=2, space="PSUM") as psB,
            tc.tile_pool(name="psC", bufs=1, space="PSUM") as psC,
            tc.tile_pool(name="dram", bufs=1, space="DRAM") as dp,
            tc.tile_pool(name="cc", bufs=4, space="DRAM") as ccp,
        ):
            w_sb = {}
            for name, dr in [("dW", dW), ("lembT", lembT)] + \
                    [(f"wihT{l}", wihT[l]) for l in range(3)] + \
                    [(f"whhT{l}", whhT[l]) for l in range(3)]:
                t = wp.tile([128, dr.shape[1]], bf16, tag=name, name=f"w_{name}")
                nc.sync.dma_start(t[:], dr[:])
                w_sb[name] = t
            db_sb = wp.tile([128, 4], f32, tag="db")
            nc.sync.dma_start(db_sb[:], db[:])
            bias_sb = {}
            for l in range(3):
                bias_sb[l] = wp.tile([128, G4[l]], f32, tag=f"bias{l}", name=f"bias_sb{l}")
                nc.sync.dma_start(bias_sb[l][:], bias[l][:])
            id_sb = wp.tile([128, 128], bf16, tag="id")
            nc.sync.dma_start(id_sb[:], ident[:])

            pre_d = {l: dp.tile([NT, G4[l]], f32, tag=f"pre{l}", name=f"pre_d{l}")
                     for l in range(3)}
            x0T_d = dp.tile([128, 4 * NT], bf16, tag="x0T")
            x1T_d = dp.tile([128, 9 * NT], bf16, tag="x1T")
            x2T_d = dp.tile([128, 9 * NT], bf16, tag="x2T")
            x3T_d = dp.tile([128, 4 * NT], bf16, tag="x3T")
            xT_d = {0: x0T_d, 1: x1T_d, 2: x2T_d}

            # ---- front end ----
            ids_sb = fe.tile([128, NMT], i32, tag="ids")
            nc.sync.dma_start(ids_sb[:], ids[:])
            for mtc in range(0, NMT, 4):
                g = fe.tile([128, 4, E], f32, tag="gchunk")
                nc.gpsimd.indirect_dma_start(
                    g[:, :, :], None, embt[:, :],
                    IndirectOffsetOnAxis(ap=ids_sb[:, mtc:mtc + 4], axis=0),
                )
                gb = fe.tile([128, 4, E], bf16, tag="gbf")
                nc.vector.tensor_copy(gb[:], g[:])
                embTc = fe.tile([128, 4, 512], bf16, tag="embTc")
                for j in range(4):
                    for ec in range(4):
                        fv = 128 if ec < 3 else 16
                        ps = psB.tile([128, 128], bf16, tag="tp")
                        nc.tensor.transpose(
                            ps[:fv, :], gb[:, j, ec * 128:ec * 128 + fv],
                            id_sb[:, :])
                        nc.vector.tensor_copy(
                            embTc[:fv, ec, j * 128:(j + 1) * 128],
                            ps[:fv, :])
                for mf in range(4):
                    ps = psC.tile([128, 512], f32, tag="bigps")
                    for ec in range(4):
                        fv = 128 if ec < 3 else 16
                        nc.tensor.matmul(
                            ps[:, :],
                            w_sb["dW"][:fv, ec * M + mf * 128:
                                       ec * M + (mf + 1) * 128],
                            embTc[:fv, ec, :],
                            start=(ec == 0), stop=(ec == 3),
                        )
                    xs = fe.tile([128, 512], f32, tag="x0drain")
                    nc.scalar.add(xs[:], ps[:], db_sb[:, mf:mf + 1])
                    xb = fe.tile([128, 512], bf16, tag="x0bf")
                    nc.vector.tensor_copy(xb[:], xs[:])
                    nc.sync.dma_start(
                        x0T_d[:, mf * NT + mtc * 128:
                              mf * NT + mtc * 128 + 512], xb[:])

            def in_gemm(l, beta):
                g4 = G4[l]
                nb = 2 if g4 > 512 else 1
                nsz = g4 // nb
                for mt in range(8 * beta, 8 * beta + 8):
                    pss = [psA.tile([128, nsz], f32, tag=f"g{l}q{q}", name=f"ig_ps{l}{q}")
                           for q in range(nb)]
                    for kt in range(KIN[l]):
                        lt = sp.tile([128, 128], bf16, tag="iglhs")
                        nc.sync.dma_start(
                            lt[:], xT_d[l][:, kt * NT + mt * 128:
                                           kt * NT + (mt + 1) * 128])
                        lhsT = lt[:]
                        for q in range(nb):
                            nc.tensor.matmul(
                                pss[q][:], lhsT,
                                w_sb[f"wihT{l}"][:, kt * g4 + q * nsz:
                                                 kt * g4 + (q + 1) * nsz],
                                start=(kt == 0), stop=(kt == KIN[l] - 1),
                            )
                    pr = sp.tile([128, g4], f32, tag="igdrain")
                    for q in range(nb):
                        nc.vector.tensor_tensor(
                            pr[:, q * nsz:(q + 1) * nsz], pss[q][:],
                            bias_sb[l][:, q * nsz:(q + 1) * nsz], Alu.add)
                    nc.sync.dma_start(
                        pre_d[l][mt * 128:(mt + 1) * 128, :], pr[:])

            cst, hT = {}, {}
            for l in range(3):
                kh, sg = KH[l], SG[l]
                cst[l] = sp.tile([32, sg], f32, tag=f"c{l}", name=f"cst{l}")
                nc.sync.dma_start(cst[l][:], c0s[l][:])
                h0f_t = sp.tile([128, kh * 32], f32, tag=f"h0T{l}")
                nc.sync.dma_start(h0f_t[:], h0T[l][:])
                hT[l] = sp.tile([128, kh, 32], bf16, tag=f"hT{l}", name=f"hT_i{l}")
                nc.vector.tensor_copy(
                    hT[l][:], h0f_t[:].rearrange("p (k b) -> p k b", b=32))

            def scan_block(l, beta):
                kh, sg, g4, khv = KH[l], SG[l], G4[l], KHV[l]
                nb = 2 if g4 > 512 else 1
                nsz = g4 // nb
                for t in range(32 * beta, 32 * beta + 32):
                    prs = sp.tile([32, g4], f32, tag=f"prs{l}")
                    nc.sync.dma_start(prs[:], pre_d[l][32 * t:32 * t + 32, :])
                    pss = [psA.tile([32, nsz], f32, tag=f"g{l}q{q}", name=f"sc_ps{l}{q}")
                           for q in range(nb)]
                    for kt in range(kh):
                        kv = 128 if kt < kh - 1 else khv
                        for q in range(nb):
                            nc.tensor.matmul(
                                pss[q][:],
                                hT[l][:kv, kt, :],
                                w_sb[f"whhT{l}"][:kv, kt * g4 + q * nsz:
                                                 kt * g4 + (q + 1) * nsz],
                                start=(kt == 0), stop=(kt == kh - 1),
                            )
                    gt = sp.tile([32, g4], f32, tag=f"gt{l}")
                    for q in range(nb):
                        nc.vector.tensor_tensor(
                            gt[:, q * nsz:(q + 1) * nsz], pss[q][:],
                            prs[:, q * nsz:(q + 1) * nsz], Alu.add)
                    gi = sp.tile([32, sg], f32, tag=f"gi{l}")
                    gf = sp.tile([32, sg], f32, tag=f"gf{l}")
                    gg = sp.tile([32, sg], f32, tag=f"gg{l}")
                    go = sp.tile([32, sg], f32, tag=f"go{l}")
                    nc.scalar.activation(gi[:], gt[:, 0:sg], AF.Sigmoid)
                    nc.scalar.activation(gf[:], gt[:, sg:2 * sg], AF.Sigmoid)
                    nc.scalar.activation(gg[:], gt[:, 2 * sg:3 * sg], AF.Tanh)
                    nc.scalar.activation(go[:], gt[:, 3 * sg:4 * sg], AF.Sigmoid)
                    t1 = sp.tile([32, sg], f32, tag=f"t1{l}")
                    nc.vector.tensor_mul(t1[:], gf[:], cst[l][:])
                    t2 = sp.tile([32, sg], f32, tag=f"t2{l}")
                    nc.vector.tensor_mul(t2[:], gi[:], gg[:])
                    nc.vector.tensor_add(cst[l][:], t1[:], t2[:])
                    tch = sp.tile([32, sg], f32, tag=f"tch{l}")
                    nc.scalar.activation(tch[:], cst[l][:], AF.Tanh)
                    hs = sp.tile([32, sg], f32, tag=f"hs{l}")
                    nc.vector.tensor_mul(hs[:], go[:], tch[:])
                    hsb = sp.tile([32, sg], bf16, tag=f"hsb{l}")
                    nc.vector.tensor_copy(hsb[:], hs[:])
                    cin = ccp.tile([32, sg], bf16, tag=f"cin{l}")
                    cout = ccp.tile([32 * NC, sg], bf16, addr_space="Shared",
                                    tag=f"cout{l}")
                    nc.sync.dma_start(cin[:], hsb[:])
                    nc.gpsimd.collective_compute(
                        "AllGather", Alu.bypass,
                        replica_groups=[list(range(NC))],
                        ins=[cin[:]], outs=[cout[:]],
                    )
                    hfull = sp.tile([32, NC, sg], bf16, tag=f"hfull{l}")
                    nc.sync.dma_start(
                        hfull[:], cout[:].rearrange("(r b) j -> b r j", b=32))
                    hT[l] = sp.tile([128, kh, 32], bf16, tag=f"hT{l}", name=f"hT_s{l}")
                    hfl = hfull[:].rearrange("b r j -> b (r j)")
                    for kt in range(kh):
                        kv = 128 if kt < kh - 1 else khv
                        ps = psB.tile([128, 128], bf16, tag="tp")
                        nc.tensor.transpose(
                            ps[:kv, :32], hfl[:, kt * 128:kt * 128 + kv],
                            id_sb[:32, :32])
                        nc.vector.tensor_copy(hT[l][:kv, kt, :],
                                              ps[:kv, :32])
                    dst = xT_d[l + 1] if l < 2 else x3T_d
                    nkt = 9 if l < 2 else 4
                    nc.sync.dma_start(
                        dst[:].rearrange("p (k n) -> p k n", k=nkt)
                        [:, :, 32 * t:32 * t + 32],
                        hT[l][:, :, :])
                    if t == T - 1:
                        nc.sync.dma_start(cf[l][:], cst[l][:])
                        nc.sync.dma_start(hf[l][:], hs[:])

            def logits_block(beta):
                for mt in range(8 * beta, 8 * beta + 8):
                    lts = []
                    for kt in range(4):
                        kv = 128 if kt < 3 else 16
                        lt = lg.tile([128, 128], bf16, tag="lglhs")
                        nc.sync.dma_start(
                            lt[:kv, :], x3T_d[:kv, kt * NT + mt * 128:
                                              kt * NT + (mt + 1) * 128])
                        lts.append(lt)
                    for nbk in range(8):
                        ps = psC.tile([128, 512], f32, tag="bigps")
                        for kt in range(4):
                            kv = 128 if kt < 3 else 16
                            nc.tensor.matmul(
                                ps[:, :500],
                                lts[kt][:kv, :],
                                w_sb["lembT"][:kv, kt * VSH + nbk * 500:
                                              kt * VSH + (nbk + 1) * 500],
                                start=(kt == 0), stop=(kt == 3),
                            )
                        ob = lg.tile([128, 500], f32, tag="lgout")
                        nc.vector.tensor_copy(ob[:], ps[:, :500])
                        nc.sync.dma_start(
                            logits[mt * 128:(mt + 1) * 128,
                                   nbk * 500:(nbk + 1) * 500], ob[:])

            in_gemm(0, 0); in_gemm(0, 1); in_gemm(0, 2); in_gemm(0, 3)
            for w in range(6):
                if w < 4:
                    scan_block(0, w)
                    in_gemm(1, w)
                if 1 <= w < 5:
                    scan_block(1, w - 1)
                    in_gemm(2, w - 1)
                if 2 <= w:
                    scan_block(2, w - 2)
                    logits_block(w - 2)

    nc.compile()
    return nc


def _host_prep(input_ids, emb_table, define_W, define_b,
               Wih0, Whh0, bih0, bhh0, Wih1, Whh1, bih1, bhh1,
               Wih2, Whh2, bih2, bhh2, h0_0, c0_0, h0_1, c0_1, h0_2, c0_2):
    ids_tm = np.ascontiguousarray(input_ids.T).astype(np.int32)  # [T, B]
    ids_pk = np.ascontiguousarray(ids_tm.reshape(NMT, 128).T)
    Wihs, Whhs = [Wih0, Wih1, Wih2], [Whh0, Whh1, Whh2]
    biases = [np.asarray(bih0) + np.asarray(bhh0),
              np.asarray(bih1) + np.asarray(bhh1),
              np.asarray(bih2) + np.asarray(bhh2)]
    h0s, c0s_ = [h0_0, h0_1, h0_2], [c0_0, c0_1, c0_2]
    Hs, Ss = [H, H, H2], [S01, S01, S2]
    db_pk = _pack_kT(np.asarray(define_b).reshape(M, 1)).reshape(128, 4)
    common = {
        "ids": ids_pk,
        "embt": np.ascontiguousarray(emb_table, np.float32),
        "dW": _bf16(_pack_kT(np.asarray(define_W))),
        "db": np.ascontiguousarray(db_pk, np.float32),
        "ident": _bf16(np.eye(128, dtype=np.float32)),
    }
    in_maps = []
    for k in range(NC):
        m = dict(common)
        m["lembT"] = _bf16(_pack_kT(np.ascontiguousarray(
            np.asarray(emb_table)[k * VSH:(k + 1) * VSH].T)))
        for l in range(3):
            sl = np.arange(k * Ss[l], (k + 1) * Ss[l])
            rows = np.concatenate([g * Hs[l] + sl for g in range(4)])
            m[f"wihT{l}"] = _bf16(_pack_kT(np.ascontiguousarray(
                np.asarray(Wihs[l])[rows].T)))
            m[f"whhT{l}"] = _bf16(_pack_kT(np.ascontiguousarray(
                np.asarray(Whhs[l])[rows].T)))
            m[f"bias{l}"] = np.ascontiguousarray(
                np.tile(biases[l][rows].reshape(1, -1), (128, 1)), np.float32)
            m[f"h0T{l}"] = _pack_kT(np.ascontiguousarray(
                np.asarray(h0s[l]).T)).astype(np.float32)
            m[f"c0s{l}"] = np.ascontiguousarray(
                np.asarray(c0s_[l])[:, sl], np.float32)
        in_maps.append(m)
    return in_maps


def kernel(**inputs):
    global _COMPILED
    from concourse import bass_utils
    if _COMPILED is None:
        _COMPILED = _build()
    nc = _COMPILED
    in_maps = _host_prep(**{k: np.asarray(v) for k, v in inputs.items()})
    res = bass_utils.run_bass_kernel_spmd(nc, in_maps, core_ids=list(range(NC)))
    r = res.results
    lg = np.concatenate([r[k]["logits"] for k in range(NC)], axis=1)
    logits = np.ascontiguousarray(
        lg.reshape(T, B, V).transpose(1, 0, 2), np.float32)
    outs = []
    for l in range(3):
        outs.append(np.concatenate([r[k][f"hf{l}"] for k in range(NC)], axis=1))
        outs.append(np.concatenate([r[k][f"cf{l}"] for k in range(NC)], axis=1))
    return (logits, outs[0], outs[1], outs[2], outs[3], outs[4], outs[5])


# revision 8
# speedup vs baseline: 1.5436x; 1.5436x over previous
"""AWD-LSTM Trainium2 kernel: 8-core SPMD, gate-sharded LSTM scan with
per-step AllGather h-exchange, vocab-sharded tied-embedding logits.

Self-contained; shapes hardcoded: B=32, T=128, V=32000, E=400, M=512,
H=1152 (layers 0,1), H2=400 (layer 2). Token order on device is t-major
(n = t*B + b) so per-layer input GEMMs block-pipeline behind the previous
layer's scan (wavefront emission).
"""
import sys
import numpy as np

sys.path.insert(0, "/opt/trn_rl_repo")

B, T, V, E, M, H = 32, 128, 32000, 400, 512, 1152
H2 = 400
NC = 8
S01 = H // NC
S2 = H2 // NC
VSH = V // NC
NT = B * T
NMT = NT // 128

_COMPILED = None


def _pack_kT(w, dtype=np.float32):
    """[K, M] -> [128, ceil(K/128)*M]; tile kt at [:, kt*M:(kt+1)*M]."""
    K, Mw = w.shape
    nkt = (K + 127) // 128
    out = np.zeros((128, nkt * Mw), dtype)
    for kt in range(nkt):
        k0, k1 = kt * 128, min(K, (kt + 1) * 128)
        out[: k1 - k0, kt * Mw:(kt + 1) * Mw] = w[k0:k1]
    return out


def _bf16(x):
    import jax.numpy as jnp
    return np.asarray(jnp.asarray(np.asarray(x), dtype=jnp.bfloat16))


def _build():
    import concourse.bass as bass
    import concourse.bacc as bacc
    import concourse.tile as tile
    from concourse import mybir
    from concourse.bass import IndirectOffsetOnAxis

    f32, bf16, i32 = mybir.dt.float32, mybir.dt.bfloat16, mybir.dt.int32
    AF = mybir.ActivationFunctionType
    Alu = mybir.AluOpType

    nc = bacc.Bacc("TRN2", target_bir_lowering=False, debug=False, num_devices=NC)

    embg = nc.dram_tensor("embg", [NT, E], f32, kind="ExternalInput")
    dW = nc.dram_tensor("dW", [128, 4 * M], bf16, kind="ExternalInput")
    db = nc.dram_tensor("db", [128, 4], f32, kind="ExternalInput")
    wihT, whhT = {}, {}
    for l, (kin, sg) in enumerate([(4, S01), (9, S01), (9, S2)]):
        wihT[l] = nc.dram_tensor(f"wihT{l}", [128, kin * 4 * sg], bf16,
                                 kind="ExternalInput")
    for l, (kh, sg) in enumerate([(9, S01), (9, S01), (4, S2)]):
        whhT[l] = nc.dram_tensor(f"whhT{l}", [128, kh * 4 * sg], bf16,
                                 kind="ExternalInput")
    bias = {l: nc.dram_tensor(f"bias{l}", [128, 4 * sg], f32,
                              kind="ExternalInput")
            for l, sg in [(0, S01), (1, S01), (2, S2)]}
    h0T = {l: nc.dram_tensor(f"h0T{l}", [128, kh * 32], f32,
                             kind="ExternalInput")
           for l, kh in [(0, 9), (1, 9), (2, 4)]}
    c0s = {l: nc.dram_tensor(f"c0s{l}", [32, sg], f32, kind="ExternalInput")
           for l, sg in [(0, S01), (1, S01), (2, S2)]}
    lembT = nc.dram_tensor("lembT", [128, 4 * VSH], bf16, kind="ExternalInput")
    ident = nc.dram_tensor("ident", [128, 128], bf16, kind="ExternalInput")

    logits = nc.dram_tensor("logits", [NT, VSH], f32, kind="ExternalOutput")
    hf = {l: nc.dram_tensor(f"hf{l}", [32, sg], f32, kind="ExternalOutput")
          for l, sg in [(0, S01), (1, S01), (2, S2)]}
    cf = {l: nc.dram_tensor(f"cf{l}", [32, sg], f32, kind="ExternalOutput")
          for l, sg in [(0, S01), (1, S01), (2, S2)]}

    KIN = {0: 4, 1: 9, 2: 9}
    KH = {0: 9, 1: 9, 2: 4}
    KHV = {0: 128, 1: 128, 2: 16}
    SG = {0: S01, 1: S01, 2: S2}
    G4 = {l: 4 * SG[l] for l in range(3)}

    with tile.TileContext(nc) as tc:
        with (
            tc.tile_pool(name="wpool", bufs=1) as wp,
            tc.tile_pool(name="fe", bufs=2) as fe,
            tc.tile_pool(name="scan", bufs=2) as sp,
            tc.tile_pool(name="lg", bufs=5) as lg,
            tc.tile_pool(name="psA", bufs=1, space="PSUM") as psA,
            tc.tile_pool(name="psB", bufs=2, space="PSUM") as psB,
            tc.tile_pool(name="psC", bufs=1, space="PSUM") as psC,
            tc.tile_pool(name="dram", bufs=1, space="DRAM") as dp,
            tc.tile_pool(name="cc", bufs=4, space="DRAM") as ccp,
        ):
            w_sb = {}
            for name, dr in [("dW", dW), ("lembT", lembT)] + \
                    [(f"wihT{l}", wihT[l]) for l in range(3)] + \
                    [(f"whhT{l}", whhT[l]) for l in range(3)]:
                t = wp.tile([128, dr.shape[1]], bf16, tag=name, name=f"w_{name}")
                nc.sync.dma_start(t[:], dr[:])
                w_sb[name] = t
            db_sb = wp.tile([128, 4], f32, tag="db")
            nc.sync.dma_start(db_sb[:], db[:])
            bias_sb = {}
            for l in range(3):
                bias_sb[l] = wp.tile([128, G4[l]], f32, tag=f"bias{l}", name=f"bias_sb{l}")
                nc.sync.dma_start(bias_sb[l][:], bias[l][:])
            id_sb = wp.tile([128, 128], bf16, tag="id")
            nc.sync.dma_start(id_sb[:], ident[:])

            pre_d = {l: dp.tile([NT, G4[l]], f32, tag=f"pre{l}", name=f"pre_d{l}")
                     for l in range(3)}
            x0T_d = dp.tile([128, 4 * NT], bf16, tag="x0T")
            x1T_d = dp.tile([128, 9 * NT], bf16, tag="x1T")
            x2T_d = dp.tile([128, 9 * NT], bf16, tag="x2T")
            x3T_d = dp.tile([128, 4 * NT], bf16, tag="x3T")
            xT_d = {0: x0T_d, 1: x1T_d, 2: x2T_d}

            # ---- front end ----
            for mtc in range(0, NMT, 4):
                g = fe.tile([128, 4, E], f32, tag="gchunk")
                nc.sync.dma_start(
                    g[:, :, :],
                    embg[mtc * 128:(mtc + 4) * 128, :]
                    .rearrange("(j p) e -> p j e", p=128))
                gb = fe.tile([128, 4, E], bf16, tag="gbf")
                nc.vector.tensor_copy(gb[:], g[:])
                embTc = fe.tile([128, 4, 512], bf16, tag="embTc")
                for j in range(4):
                    for ec in range(4):
                        fv = 128 if ec < 3 else 16
                        ps = psB.tile([128, 128], bf16, tag="tp")
                        nc.tensor.transpose(
                            ps[:fv, :], gb[:, j, ec * 128:ec * 128 + fv],
                            id_sb[:, :])
                        nc.vector.tensor_copy(
                            embTc[:fv, ec, j * 128:(j + 1) * 128],
                            ps[:fv, :])
                for mf in range(4):
                    ps = psC.tile([128, 512], f32, tag="bigps")
                    for ec in range(4):
                        fv = 128 if ec < 3 else 16
                        nc.tensor.matmul(
                            ps[:, :],
                            w_sb["dW"][:fv, ec * M + mf * 128:
                                       ec * M + (mf + 1) * 128],
                            embTc[:fv, ec, :],
                            start=(ec == 0), stop=(ec == 3),
                        )
                    xs = fe.tile([128, 512], f32, tag="x0drain")
                    nc.scalar.add(xs[:], ps[:], db_sb[:, mf:mf + 1])
                    xb = fe.tile([128, 512], bf16, tag="x0bf")
                    nc.vector.tensor_copy(xb[:], xs[:])
                    nc.sync.dma_start(
                        x0T_d[:, mf * NT + mtc * 128:
                              mf * NT + mtc * 128 + 512], xb[:])

            def in_gemm(l, beta):
                g4 = G4[l]
                nb = 2 if g4 > 512 else 1
                nsz = g4 // nb
                for mt in range(8 * beta, 8 * beta + 8):
                    pss = [psA.tile([128, nsz], f32, tag=f"g{l}q{q}", name=f"ig_ps{l}{q}")
                           for q in range(nb)]
                    for kt in range(KIN[l]):
                        lt = sp.tile([128, 128], bf16, tag="iglhs")
                        nc.sync.dma_start(
                            lt[:], xT_d[l][:, kt * NT + mt * 128:
                                           kt * NT + (mt + 1) * 128])
                        lhsT = lt[:]
                        for q in range(nb):
                            nc.tensor.matmul(
                                pss[q][:], lhsT,
                                w_sb[f"wihT{l}"][:, kt * g4 + q * nsz:
                                                 kt * g4 + (q + 1) * nsz],
                                start=(kt == 0), stop=(kt == KIN[l] - 1),
                            )
                    pr = sp.tile([128, g4], f32, tag="igdrain")
                    for q in range(nb):
                        nc.vector.tensor_tensor(
                            pr[:, q * nsz:(q + 1) * nsz], pss[q][:],
                            bias_sb[l][:, q * nsz:(q + 1) * nsz], Alu.add)
                    nc.sync.dma_start(
                        pre_d[l][mt * 128:(mt + 1) * 128, :], pr[:])

            cst, hT = {}, {}
            for l in range(3):
                kh, sg = KH[l], SG[l]
                cst[l] = sp.tile([32, sg], f32, tag=f"c{l}", name=f"cst{l}")
                nc.sync.dma_start(cst[l][:], c0s[l][:])
                h0f_t = sp.tile([128, kh * 32], f32, tag=f"h0T{l}")
                nc.sync.dma_start(h0f_t[:], h0T[l][:])
                hT[l] = sp.tile([128, kh, 32], bf16, tag=f"hT{l}", name=f"hT_i{l}")
                nc.vector.tensor_copy(
                    hT[l][:], h0f_t[:].rearrange("p (k b) -> p k b", b=32))

            def scan_block(l, beta):
                kh, sg, g4, khv = KH[l], SG[l], G4[l], KHV[l]
                nb = 2 if g4 > 512 else 1
                nsz = g4 // nb
                for t in range(32 * beta, 32 * beta + 32):
                    prs = sp.tile([32, g4], f32, tag=f"prs{l}")
                    nc.sync.dma_start(prs[:], pre_d[l][32 * t:32 * t + 32, :])
                    pss = [psA.tile([32, nsz], f32, tag=f"g{l}q{q}", name=f"sc_ps{l}{q}")
                           for q in range(nb)]
                    for kt in range(kh):
                        kv = 128 if kt < kh - 1 else khv
                        for q in range(nb):
                            nc.tensor.matmul(
                                pss[q][:],
                                hT[l][:kv, kt, :],
                                w_sb[f"whhT{l}"][:kv, kt * g4 + q * nsz:
                                                 kt * g4 + (q + 1) * nsz],
                                start=(kt == 0), stop=(kt == kh - 1),
                            )
                    gt = sp.tile([32, g4], f32, tag=f"gt{l}")
                    for q in range(nb):
                        nc.vector.tensor_tensor(
                            gt[:, q * nsz:(q + 1) * nsz], pss[q][:],
                            prs[:, q * nsz:(q + 1) * nsz], Alu.add)
                    gi = sp.tile([32, sg], f32, tag=f"gi{l}")
                    gf = sp.tile([32, sg], f32, tag=f"gf{l}")
                    gg = sp.tile([32, sg], f32, tag=f"gg{l}")
                    go = sp.tile([32, sg], f32, tag=f"go{l}")
                    nc.scalar.activation(gi[:], gt[:, 0:sg], AF.Sigmoid)
                    nc.scalar.activation(gf[:], gt[:, sg:2 * sg], AF.Sigmoid)
                    nc.scalar.activation(gg[:], gt[:, 2 * sg:3 * sg], AF.Tanh)
                    nc.scalar.activation(go[:], gt[:, 3 * sg:4 * sg], AF.Sigmoid)
                    t1 = sp.tile([32, sg], f32, tag=f"t1{l}")
                    nc.vector.tensor_mul(t1[:], gf[:], cst[l][:])
                    t2 = sp.tile([32, sg], f32, tag=f"t2{l}")
                    nc.vector.tensor_mul(t2[:], gi[:], gg[:])
                    nc.vector.tensor_add(cst[l][:], t1[:], t2[:])
                    tch = sp.tile([32, sg], f32, tag=f"tch{l}")
                    nc.scalar.activation(tch[:], cst[l][:], AF.Tanh)
                    hs = sp.tile([32, sg], f32, tag=f"hs{l}")
                    nc.vector.tensor_mul(hs[:], go[:], tch[:])
                    hsb = sp.tile([32, sg], bf16, tag=f"hsb{l}")
                    nc.vector.tensor_copy(hsb[:], hs[:])
                    cin = ccp.tile([32, sg], bf16, tag=f"cin{l}")
                    cout = ccp.tile([32 * NC, sg], bf16, addr_space="Shared",
                                    tag=f"cout{l}")
                    nc.sync.dma_start(cin[:], hsb[:])
                    nc.gpsimd.collective_compute(
                        "AllGather", Alu.bypass,
                        replica_groups=[list(range(NC))],
                        ins=[cin[:]], outs=[cout[:]],
                    )
                    hfull = sp.tile([32, NC, sg], bf16, tag=f"hfull{l}")
                    nc.sync.dma_start(
                        hfull[:], cout[:].rearrange("(r b) j -> b r j", b=32))
                    hT[l] = sp.tile([128, kh, 32], bf16, tag=f"hT{l}", name=f"hT_s{l}")
                    hfl = hfull[:].rearrange("b r j -> b (r j)")
                    for kt in range(kh):
                        kv = 128 if kt < kh - 1 else khv
                        ps = psB.tile([128, 128], bf16, tag="tp")
                        nc.tensor.transpose(
                            ps[:kv, :32], hfl[:, kt * 128:kt * 128 + kv],
                            id_sb[:32, :32])
                        nc.vector.tensor_copy(hT[l][:kv, kt, :],
                                              ps[:kv, :32])
                    dst = xT_d[l + 1] if l < 2 else x3T_d
                    nkt = 9 if l < 2 else 4
                    nc.sync.dma_start(
                        dst[:].rearrange("p (k n) -> p k n", k=nkt)
                        [:, :, 32 * t:32 * t + 32],
                        hT[l][:, :, :])
                    if t == T - 1:
                        nc.sync.dma_start(cf[l][:], cst[l][:])
                        nc.sync.dma_start(hf[l][:], hs[:])

            def logits_block(beta):
                for mt in range(8 * beta, 8 * beta + 8):
                    lts = []
                    for kt in range(4):
                        kv = 128 if kt < 3 else 16
                        lt = lg.tile([128, 128], bf16, tag="lglhs")
                        nc.sync.dma_start(
                            lt[:kv, :], x3T_d[:kv, kt * NT + mt * 128:
                                              kt * NT + (mt + 1) * 128])
                        lts.append(lt)
                    for nbk in range(8):
                        ps = psC.tile([128, 512], f32, tag="bigps")
                        for kt in range(4):
                            kv = 128 if kt < 3 else 16
                            nc.tensor.matmul(
                                ps[:, :500],
                                lts[kt][:kv, :],
                                w_sb["lembT"][:kv, kt * VSH + nbk * 500:
                                              kt * VSH + (nbk + 1) * 500],
                                start=(kt == 0), stop=(kt == 3),
                            )
                        ob = lg.tile([128, 500], f32, tag="lgout")
                        nc.vector.tensor_copy(ob[:], ps[:, :500])
                        nc.sync.dma_start(
                            logits[mt * 128:(mt + 1) * 128,
                                   nbk * 500:(nbk + 1) * 500], ob[:])

            in_gemm(0, 0); in_gemm(0, 1); in_gemm(0, 2); in_gemm(0, 3)
            for w in range(6):
                if w < 4:
                    scan_block(0, w)
                    in_gemm(1, w)
                if 1 <= w < 5:
                    scan_block(1, w - 1)
                    in_gemm(2, w - 1)
                if 2 <= w:
                    scan_block(2, w - 2)
                    logits_block(w - 2)

    nc.compile()
    return nc


def _host_prep(input_ids, emb_table, define_W, define_b,
               Wih0, Whh0, bih0, bhh0, Wih1, Whh1, bih1, bhh1,
               Wih2, Whh2, bih2, bhh2, h0_0, c0_0, h0_1, c0_1, h0_2, c0_2):
    ids_tm = np.ascontiguousarray(input_ids.T).astype(np.int32)  # [T, B]
    ids_pk = np.ascontiguousarray(ids_tm.reshape(NMT, 128).T)
    Wihs, Whhs = [Wih0, Wih1, Wih2], [Whh0, Whh1, Whh2]
    biases = [np.asarray(bih0) + np.asarray(bhh0),
              np.asarray(bih1) + np.asarray(bhh1),
              np.asarray(bih2) + np.asarray(bhh2)]
    h0s, c0s_ = [h0_0, h0_1, h0_2], [c0_0, c0_1, c0_2]
    Hs, Ss = [H, H, H2], [S01, S01, S2]
    db_pk = _pack_kT(np.asarray(define_b).reshape(M, 1)).reshape(128, 4)
    embg = np.ascontiguousarray(
        np.asarray(emb_table)[ids_tm.reshape(-1)], np.float32)
    common = {
        "embg": embg,
        "dW": _bf16(_pack_kT(np.asarray(define_W))),
        "db": np.ascontiguousarray(db_pk, np.float32),
        "ident": _bf16(np.eye(128, dtype=np.float32)),
    }
    in_maps = []
    for k in range(NC):
        m = dict(common)
        m["lembT"] = _bf16(_pack_kT(np.ascontiguousarray(
            np.asarray(emb_table)[k * VSH:(k + 1) * VSH].T)))
        for l in range(3):
            sl = np.arange(k * Ss[l], (k + 1) * Ss[l])
            rows = np.concatenate([g * Hs[l] + sl for g in range(4)])
            m[f"wihT{l}"] = _bf16(_pack_kT(np.ascontiguousarray(
                np.asarray(Wihs[l])[rows].T)))
            m[f"whhT{l}"] = _bf16(_pack_kT(np.ascontiguousarray(
                np.asarray(Whhs[l])[rows].T)))
            m[f"bias{l}"] = np.ascontiguousarray(
                np.tile(biases[l][rows].reshape(1, -1), (128, 1)), np.float32)
            m[f"h0T{l}"] = _pack_kT(np.ascontiguousarray(
                np.asarray(h0s[l]).T)).astype(np.float32)
            m[f"c0s{l}"] = np.ascontiguousarray(
                np.asarray(c0s_[l])[:, sl], np.float32)
        in_maps.append(m)
    return in_maps


def kernel(**inputs):
    global _COMPILED
    from concourse import bass_utils
    if _COMPILED is None:
        _COMPILED = _build()
    nc = _COMPILED
    in_maps = _host_prep(**{k: np.asarray(v) for k, v in inputs.items()})
    res = bass_utils.run_bass_kernel_spmd(nc, in_maps, core_ids=list(range(NC)))
    r = res.results
    lg = np.concatenate([r[k]["logits"] for k in range(NC)], axis=1)
    logits = np.ascontiguousarray(
        lg.reshape(T, B, V).transpose(1, 0, 2), np.float32)
    outs = []
    for l in range(3):
        outs.append(np.concatenate([r[k][f"hf{l}"] for k in range(NC)], axis=1))
        outs.append(np.concatenate([r[k][f"cf{l}"] for k in range(NC)], axis=1))
    return (logits, outs[0], outs[1], outs[2], outs[3], outs[4], outs[5])


# revision 9
# speedup vs baseline: 1.8856x; 1.2215x over previous
"""AWD-LSTM Trainium2 kernel: 8-core SPMD, gate-sharded LSTM scan with
per-step AllGather h-exchange, vocab-sharded tied-embedding logits.

Self-contained; shapes hardcoded: B=32, T=128, V=32000, E=400, M=512,
H=1152 (layers 0,1), H2=400 (layer 2). Token order on device is t-major
(n = t*B + b) so per-layer input GEMMs block-pipeline behind the previous
layer's scan (wavefront emission).
"""
import sys
import numpy as np

sys.path.insert(0, "/opt/trn_rl_repo")

B, T, V, E, M, H = 32, 128, 32000, 400, 512, 1152
H2 = 400
NC = 8
S01 = H // NC
S2 = H2 // NC
VSH = V // NC
NT = B * T
NMT = NT // 128

_COMPILED = None


def _pack_kT(w, dtype=np.float32):
    """[K, M] -> [128, ceil(K/128)*M]; tile kt at [:, kt*M:(kt+1)*M]."""
    K, Mw = w.shape
    nkt = (K + 127) // 128
    out = np.zeros((128, nkt * Mw), dtype)
    for kt in range(nkt):
        k0, k1 = kt * 128, min(K, (kt + 1) * 128)
        out[: k1 - k0, kt * Mw:(kt + 1) * Mw] = w[k0:k1]
    return out


def _bf16(x):
    import jax.numpy as jnp
    return np.asarray(jnp.asarray(np.asarray(x), dtype=jnp.bfloat16))


def _build():
    import concourse.bass as bass
    import concourse.bacc as bacc
    import concourse.tile as tile
    from concourse import mybir
    from concourse.bass import IndirectOffsetOnAxis

    f32, bf16, i32 = mybir.dt.float32, mybir.dt.bfloat16, mybir.dt.int32
    AF = mybir.ActivationFunctionType
    Alu = mybir.AluOpType

    nc = bacc.Bacc("TRN2", target_bir_lowering=False, debug=False, num_devices=NC)

    embg = nc.dram_tensor("embg", [NT, E], f32, kind="ExternalInput")
    dW = nc.dram_tensor("dW", [128, 4 * M], bf16, kind="ExternalInput")
    db = nc.dram_tensor("db", [128, 4], f32, kind="ExternalInput")
    wihT, whhT = {}, {}
    for l, (kin, sg) in enumerate([(4, S01), (9, S01), (9, S2)]):
        wihT[l] = nc.dram_tensor(f"wihT{l}", [128, kin * 4 * sg], bf16,
                                 kind="ExternalInput")
    for l, (kh, sg) in enumerate([(9, S01), (9, S01), (4, S2)]):
        whhT[l] = nc.dram_tensor(f"whhT{l}", [128, kh * 4 * sg], bf16,
                                 kind="ExternalInput")
    bias = {l: nc.dram_tensor(f"bias{l}", [128, 4 * sg], f32,
                              kind="ExternalInput")
            for l, sg in [(0, S01), (1, S01), (2, S2)]}
    h0T = {l: nc.dram_tensor(f"h0T{l}", [128, kh * 32], f32,
                             kind="ExternalInput")
           for l, kh in [(0, 9), (1, 9), (2, 4)]}
    c0s = {l: nc.dram_tensor(f"c0s{l}", [32, sg], f32, kind="ExternalInput")
           for l, sg in [(0, S01), (1, S01), (2, S2)]}
    lembT = nc.dram_tensor("lembT", [128, 4 * VSH], bf16, kind="ExternalInput")
    ident = nc.dram_tensor("ident", [128, 128], bf16, kind="ExternalInput")

    logits = nc.dram_tensor("logits", [NT, VSH], f32, kind="ExternalOutput")
    hf = {l: nc.dram_tensor(f"hf{l}", [32, sg], f32, kind="ExternalOutput")
          for l, sg in [(0, S01), (1, S01), (2, S2)]}
    cf = {l: nc.dram_tensor(f"cf{l}", [32, sg], f32, kind="ExternalOutput")
          for l, sg in [(0, S01), (1, S01), (2, S2)]}

    KIN = {0: 4, 1: 9, 2: 9}
    KH = {0: 9, 1: 9, 2: 4}
    KHV = {0: 128, 1: 128, 2: 16}
    SG = {0: S01, 1: S01, 2: S2}
    G4 = {l: 4 * SG[l] for l in range(3)}

    with tile.TileContext(nc) as tc:
        with (
            tc.tile_pool(name="wpool", bufs=1) as wp,
            tc.tile_pool(name="fe", bufs=2) as fe,
            tc.tile_pool(name="scan", bufs=2) as sp,
            tc.tile_pool(name="lg", bufs=5) as lg,
            tc.tile_pool(name="psA", bufs=1, space="PSUM") as psA,
            tc.tile_pool(name="psB", bufs=2, space="PSUM") as psB,
            tc.tile_pool(name="psC", bufs=1, space="PSUM") as psC,
            tc.tile_pool(name="dram", bufs=1, space="DRAM") as dp,
            tc.tile_pool(name="cc", bufs=4, space="DRAM") as ccp,
        ):
            w_sb = {}
            for name, dr in [("dW", dW), ("lembT", lembT)] + \
                    [(f"wihT{l}", wihT[l]) for l in range(3)] + \
                    [(f"whhT{l}", whhT[l]) for l in range(3)]:
                t = wp.tile([128, dr.shape[1]], bf16, tag=name, name=f"w_{name}")
                nc.sync.dma_start(t[:], dr[:])
                w_sb[name] = t
            db_sb = wp.tile([128, 4], f32, tag="db")
            nc.sync.dma_start(db_sb[:], db[:])
            bias_sb = {}
            for l in range(3):
                bias_sb[l] = wp.tile([128, G4[l]], f32, tag=f"bias{l}", name=f"bias_sb{l}")
                nc.sync.dma_start(bias_sb[l][:], bias[l][:])
            id_sb = wp.tile([128, 128], bf16, tag="id")
            nc.sync.dma_start(id_sb[:], ident[:])

            pre_d = {l: dp.tile([NT, G4[l]], f32, tag=f"pre{l}", name=f"pre_d{l}")
                     for l in range(3)}
            x0T_d = dp.tile([128, 4 * NT], bf16, tag="x0T")
            x1T_d = dp.tile([128, 9 * NT], bf16, tag="x1T")
            x2T_d = dp.tile([128, 9 * NT], bf16, tag="x2T")
            x3T_d = dp.tile([128, 4 * NT], bf16, tag="x3T")
            xT_d = {0: x0T_d, 1: x1T_d, 2: x2T_d}

            # ---- front end ----
            for mtc in range(0, NMT, 4):
                g = fe.tile([128, 4, E], f32, tag="gchunk")
                nc.sync.dma_start(
                    g[:, :, :],
                    embg[mtc * 128:(mtc + 4) * 128, :]
                    .rearrange("(j p) e -> p j e", p=128))
                gb = fe.tile([128, 4, E], bf16, tag="gbf")
                nc.vector.tensor_copy(gb[:], g[:])
                embTc = fe.tile([128, 4, 512], bf16, tag="embTc")
                for j in range(4):
                    for ec in range(4):
                        fv = 128 if ec < 3 else 16
                        ps = psB.tile([128, 128], bf16, tag="tp")
                        nc.tensor.transpose(
                            ps[:fv, :], gb[:, j, ec * 128:ec * 128 + fv],
                            id_sb[:, :])
                        nc.vector.tensor_copy(
                            embTc[:fv, ec, j * 128:(j + 1) * 128],
                            ps[:fv, :])
                for mf in range(4):
                    ps = psC.tile([128, 512], f32, tag="bigps")
                    for ec in range(4):
                        fv = 128 if ec < 3 else 16
                        nc.tensor.matmul(
                            ps[:, :],
                            w_sb["dW"][:fv, ec * M + mf * 128:
                                       ec * M + (mf + 1) * 128],
                            embTc[:fv, ec, :],
                            start=(ec == 0), stop=(ec == 3),
                        )
                    xs = fe.tile([128, 512], f32, tag="x0drain")
                    nc.scalar.add(xs[:], ps[:], db_sb[:, mf:mf + 1])
                    xb = fe.tile([128, 512], bf16, tag="x0bf")
                    nc.vector.tensor_copy(xb[:], xs[:])
                    nc.sync.dma_start(
                        x0T_d[:, mf * NT + mtc * 128:
                              mf * NT + mtc * 128 + 512], xb[:])

            def in_gemm(l, beta):
                g4 = G4[l]
                nb = 2 if g4 > 512 else 1
                nsz = g4 // nb
                for mt in range(8 * beta, 8 * beta + 8):
                    pss = [psA.tile([128, nsz], f32, tag=f"g{l}q{q}", name=f"ig_ps{l}{q}")
                           for q in range(nb)]
                    for kt in range(KIN[l]):
                        lt = sp.tile([128, 128], bf16, tag="iglhs")
                        nc.sync.dma_start(
                            lt[:], xT_d[l][:, kt * NT + mt * 128:
                                           kt * NT + (mt + 1) * 128])
                        lhsT = lt[:]
                        for q in range(nb):
                            nc.tensor.matmul(
                                pss[q][:], lhsT,
                                w_sb[f"wihT{l}"][:, kt * g4 + q * nsz:
                                                 kt * g4 + (q + 1) * nsz],
                                start=(kt == 0), stop=(kt == KIN[l] - 1),
                            )
                    pr = sp.tile([128, g4], f32, tag="igdrain")
                    for q in range(nb):
                        nc.vector.tensor_tensor(
                            pr[:, q * nsz:(q + 1) * nsz], pss[q][:],
                            bias_sb[l][:, q * nsz:(q + 1) * nsz], Alu.add)
                    nc.sync.dma_start(
                        pre_d[l][mt * 128:(mt + 1) * 128, :], pr[:])

            cst, hT = {}, {}
            for l in range(3):
                kh, sg = KH[l], SG[l]
                cst[l] = sp.tile([32, sg], f32, tag=f"c{l}", name=f"cst{l}")
                nc.sync.dma_start(cst[l][:], c0s[l][:])
                h0f_t = sp.tile([128, kh * 32], f32, tag=f"h0T{l}")
                nc.sync.dma_start(h0f_t[:], h0T[l][:])
                hT[l] = sp.tile([128, kh, 32], bf16, tag=f"hT{l}", name=f"hT_i{l}")
                nc.vector.tensor_copy(
                    hT[l][:], h0f_t[:].rearrange("p (k b) -> p k b", b=32))

            def scan_block(l, beta):
                kh, sg, g4, khv = KH[l], SG[l], G4[l], KHV[l]
                nb = 2 if g4 > 512 else 1
                nsz = g4 // nb
                for t in range(32 * beta, 32 * beta + 32):
                    prs = sp.tile([32, g4], f32, tag=f"prs{l}")
                    nc.sync.dma_start(prs[:], pre_d[l][32 * t:32 * t + 32, :])
                    pss = [psA.tile([32, nsz], f32, tag=f"g{l}q{q}", name=f"sc_ps{l}{q}")
                           for q in range(nb)]
                    for kt in range(kh):
                        kv = 128 if kt < kh - 1 else khv
                        for q in range(nb):
                            nc.tensor.matmul(
                                pss[q][:],
                                hT[l][:kv, kt, :],
                                w_sb[f"whhT{l}"][:kv, kt * g4 + q * nsz:
                                                 kt * g4 + (q + 1) * nsz],
                                start=(kt == 0), stop=(kt == kh - 1),
                            )
                    gt = sp.tile([32, g4], f32, tag=f"gt{l}")
                    for q in range(nb):
                        nc.vector.tensor_tensor(
                            gt[:, q * nsz:(q + 1) * nsz], pss[q][:],
                            prs[:, q * nsz:(q + 1) * nsz], Alu.add)
                    gi = sp.tile([32, sg], f32, tag=f"gi{l}")
                    gf = sp.tile([32, sg], f32, tag=f"gf{l}")
                    gg = sp.tile([32, sg], f32, tag=f"gg{l}")
                    go = sp.tile([32, sg], f32, tag=f"go{l}")
                    nc.scalar.activation(gi[:], gt[:, 0:sg], AF.Sigmoid)
                    nc.scalar.activation(gf[:], gt[:, sg:2 * sg], AF.Sigmoid)
                    nc.scalar.activation(gg[:], gt[:, 2 * sg:3 * sg], AF.Tanh)
                    nc.scalar.activation(go[:], gt[:, 3 * sg:4 * sg], AF.Sigmoid)
                    t1 = sp.tile([32, sg], f32, tag=f"t1{l}")
                    nc.vector.tensor_mul(t1[:], gf[:], cst[l][:])
                    t2 = sp.tile([32, sg], f32, tag=f"t2{l}")
                    nc.vector.tensor_mul(t2[:], gi[:], gg[:])
                    nc.vector.tensor_add(cst[l][:], t1[:], t2[:])
                    tch = sp.tile([32, sg], f32, tag=f"tch{l}")
                    nc.scalar.activation(tch[:], cst[l][:], AF.Tanh)
                    hs = sp.tile([32, sg], f32, tag=f"hs{l}")
                    nc.vector.tensor_mul(hs[:], go[:], tch[:])
                    hsb = sp.tile([32, sg], bf16, tag=f"hsb{l}")
                    nc.vector.tensor_copy(hsb[:], hs[:])
                    cin = ccp.tile([32, sg], bf16, tag=f"cin{l}")
                    cout = ccp.tile([32 * NC, sg], bf16, addr_space="Shared",
                                    tag=f"cout{l}")
                    nc.sync.dma_start(cin[:], hsb[:])
                    nc.gpsimd.collective_compute(
                        "AllGather", Alu.bypass,
                        replica_groups=[list(range(NC))],
                        ins=[cin[:]], outs=[cout[:]],
                    )
                    hfull = sp.tile([32, NC, sg], bf16, tag=f"hfull{l}")
                    nc.sync.dma_start(
                        hfull[:], cout[:].rearrange("(r b) j -> b r j", b=32))
                    hT[l] = sp.tile([128, kh, 32], bf16, tag=f"hT{l}", name=f"hT_s{l}")
                    hfl = hfull[:].rearrange("b r j -> b (r j)")
                    for kt in range(kh):
                        kv = 128 if kt < kh - 1 else khv
                        ps = psB.tile([128, 128], bf16, tag="tp")
                        nc.tensor.transpose(
                            ps[:kv, :32], hfl[:, kt * 128:kt * 128 + kv],
                            id_sb[:32, :32])
                        nc.vector.tensor_copy(hT[l][:kv, kt, :],
                                              ps[:kv, :32])
                    dst = xT_d[l + 1] if l < 2 else x3T_d
                    nkt = 9 if l < 2 else 4
                    nc.sync.dma_start(
                        dst[:].rearrange("p (k n) -> p k n", k=nkt)
                        [:, :, 32 * t:32 * t + 32],
                        hT[l][:, :, :])
                    if t == T - 1:
                        nc.sync.dma_start(cf[l][:], cst[l][:])
                        nc.sync.dma_start(hf[l][:], hs[:])

            def logits_block(beta):
                for mt in range(8 * beta, 8 * beta + 8):
                    lts = []
                    for kt in range(4):
                        kv = 128 if kt < 3 else 16
                        lt = lg.tile([128, 128], bf16, tag="lglhs")
                        nc.sync.dma_start(
                            lt[:kv, :], x3T_d[:kv, kt * NT + mt * 128:
                                              kt * NT + (mt + 1) * 128])
                        lts.append(lt)
                    for nbk in range(8):
                        ps = psC.tile([128, 512], f32, tag="bigps")
                        for kt in range(4):
                            kv = 128 if kt < 3 else 16
                            nc.tensor.matmul(
                                ps[:, :500],
                                lts[kt][:kv, :],
                                w_sb["lembT"][:kv, kt * VSH + nbk * 500:
                                              kt * VSH + (nbk + 1) * 500],
                                start=(kt == 0), stop=(kt == 3),
                            )
                        ob = lg.tile([128, 500], f32, tag="lgout")
                        nc.vector.tensor_copy(ob[:], ps[:, :500])
                        nc.sync.dma_start(
                            logits[mt * 128:(mt + 1) * 128,
                                   nbk * 500:(nbk + 1) * 500], ob[:])

            in_gemm(0, 0); in_gemm(0, 1); in_gemm(0, 2); in_gemm(0, 3)
            for w in range(6):
                if w < 4:
                    scan_block(0, w)
                    in_gemm(1, w)
                if 1 <= w < 5:
                    scan_block(1, w - 1)
                    in_gemm(2, w - 1)
                if 2 <= w:
                    scan_block(2, w - 2)
                    logits_block(w - 2)

    nc.compile()
    return nc



def _make_runner(nc, n_cores):
    import jax
    from jax.sharding import Mesh, PartitionSpec
    from jax.experimental.shard_map import shard_map
    from concourse import mybir
    from concourse.bass2jax import (_bass_exec_p, install_neuronx_cc_hook,
                                    partition_id_tensor)
    install_neuronx_cc_hook()
    partition_name = nc.partition_id_tensor.name if nc.partition_id_tensor else None
    in_names, out_names, out_avals, zero_outs = [], [], [], []
    for alloc in nc.m.functions[0].allocations:
        if not isinstance(alloc, mybir.MemoryLocationSet):
            continue
        name = alloc.memorylocations[0].name
        if alloc.kind == "ExternalInput":
            if name != partition_name:
                in_names.append(name)
        elif alloc.kind == "ExternalOutput":
            shape = tuple(alloc.tensor_shape)
            dtype = mybir.dt.np(alloc.dtype)
            out_avals.append(jax.core.ShapedArray(shape, dtype))
            out_names.append(name)
            zero_outs.append(np.zeros(shape, dtype))
    n_params, n_outs = len(in_names), len(out_avals)
    all_in = list(in_names) + list(out_names)
    if partition_name is not None:
        all_in.append(partition_name)

    def _body(*args):
        operands = list(args)
        if partition_name is not None:
            operands.append(partition_id_tensor())
        return tuple(_bass_exec_p.bind(
            *operands, out_avals=tuple(out_avals), in_names=tuple(all_in),
            out_names=tuple(out_names), lowering_input_output_aliases=(),
            sim_require_finite=False, sim_require_nnan=False, nc=nc))

    devices = jax.devices()[:n_cores]
    mesh = Mesh(np.asarray(devices), ("core",))
    fn = jax.jit(shard_map(
        _body, mesh=mesh, in_specs=(PartitionSpec("core"),) * (n_params + n_outs),
        out_specs=(PartitionSpec("core"),) * n_outs, check_rep=False))
    gz = [np.concatenate([z] * n_cores, axis=0) for z in zero_outs]

    def run(in_maps):
        gi = [np.concatenate([np.asarray(in_maps[c][n]) for c in range(n_cores)],
                             axis=0) for n in in_names]
        outs = [np.asarray(o) for o in fn(*gi, *gz)]
        res = []
        for c in range(n_cores):
            d = {}
            for i, n in enumerate(out_names):
                per = outs[i].shape[0] // n_cores
                d[n] = outs[i][c * per:(c + 1) * per]
            res.append(d)
        return res
    return run


def _host_prep(input_ids, emb_table, define_W, define_b,
               Wih0, Whh0, bih0, bhh0, Wih1, Whh1, bih1, bhh1,
               Wih2, Whh2, bih2, bhh2, h0_0, c0_0, h0_1, c0_1, h0_2, c0_2):
    ids_tm = np.ascontiguousarray(input_ids.T).astype(np.int32)  # [T, B]
    ids_pk = np.ascontiguousarray(ids_tm.reshape(NMT, 128).T)
    Wihs, Whhs = [Wih0, Wih1, Wih2], [Whh0, Whh1, Whh2]
    biases = [np.asarray(bih0) + np.asarray(bhh0),
              np.asarray(bih1) + np.asarray(bhh1),
              np.asarray(bih2) + np.asarray(bhh2)]
    h0s, c0s_ = [h0_0, h0_1, h0_2], [c0_0, c0_1, c0_2]
    Hs, Ss = [H, H, H2], [S01, S01, S2]
    db_pk = _pack_kT(np.asarray(define_b).reshape(M, 1)).reshape(128, 4)
    embg = np.ascontiguousarray(
        np.asarray(emb_table)[ids_tm.reshape(-1)], np.float32)
    common = {
        "embg": embg,
        "dW": _bf16(_pack_kT(np.asarray(define_W))),
        "db": np.ascontiguousarray(db_pk, np.float32),
        "ident": _bf16(np.eye(128, dtype=np.float32)),
    }
    in_maps = []
    for k in range(NC):
        m = dict(common)
        m["lembT"] = _bf16(_pack_kT(np.ascontiguousarray(
            np.asarray(emb_table)[k * VSH:(k + 1) * VSH].T)))
        for l in range(3):
            sl = np.arange(k * Ss[l], (k + 1) * Ss[l])
            rows = np.concatenate([g * Hs[l] + sl for g in range(4)])
            m[f"wihT{l}"] = _bf16(_pack_kT(np.ascontiguousarray(
                np.asarray(Wihs[l])[rows].T)))
            m[f"whhT{l}"] = _bf16(_pack_kT(np.ascontiguousarray(
                np.asarray(Whhs[l])[rows].T)))
            m[f"bias{l}"] = np.ascontiguousarray(
                np.tile(biases[l][rows].reshape(1, -1), (128, 1)), np.float32)
            m[f"h0T{l}"] = _pack_kT(np.ascontiguousarray(
                np.asarray(h0s[l]).T)).astype(np.float32)
            m[f"c0s{l}"] = np.ascontiguousarray(
                np.asarray(c0s_[l])[:, sl], np.float32)
        in_maps.append(m)
    return in_maps


def kernel(**inputs):
    global _COMPILED
    if _COMPILED is None:
        nc = _build()
        _COMPILED = _make_runner(nc, NC)
    in_maps = _host_prep(**{k: np.asarray(v) for k, v in inputs.items()})
    r = _COMPILED(in_maps)
    lg = np.concatenate([r[k]["logits"] for k in range(NC)], axis=1)
    logits = np.ascontiguousarray(
        lg.reshape(T, B, V).transpose(1, 0, 2), np.float32)
    outs = []
    for l in range(3):
        outs.append(np.concatenate([r[k][f"hf{l}"] for k in range(NC)], axis=1))
        outs.append(np.concatenate([r[k][f"cf{l}"] for k in range(NC)], axis=1))
    return (logits, outs[0], outs[1], outs[2], outs[3], outs[4], outs[5])
